# revision 69
# baseline (speedup 1.0000x reference)
"""Trainium2 Bass kernel for the vq_codebook problem.

Computes, per batch b (B=32, d=512, n=4096, r=64, T=10, 3 steps):
    D = normalize(D_init, dim=d)
    repeat 3x: Dn = normalize(D); cos = Dn^T @ normalize(X, dim=d);
               C = softmax(cos / T, over r); D = X @ C^T   (normalize-invariant
               scale factors like the per-codeword count division cancel)
    Xbar = normalize(D) @ C of the last step.

Sharding: pure batch parallelism, 4 batches per NeuronCore across 8 cores.

Final strategy (448.5us -> 302.9us on the TimelineSim cost model):
  - cos^T computed DIRECTLY per n-chunk (X chunk as the stationary matmul
    operand, Dn moving, 64-wide output): no [r,n]->[n,r] transposes and no
    cos PSUM->SBUF staging copy at all.
  - softmax runs in a packed [n | r(stride 8), ci(stride 1)] SBUF layout
    (written by the exp) so the 1/denominator multiply keeps a stride-1
    last dim (DVE 2x mode); the per-n logit scale folds into the one
    PSUM-sourced multiply.
  - X loaded natural fp32, cast to bf16 (split DVE/GPSIMD/ACT),
    PE-transposed in bf16 chunk-PAIRS into [128,1024] xt tiles (one wide
    PSUM->SBUF copy per pair); per-n sum-squares split 1:3 DVE:ACT.
  - XCt accumulates all 32 n-chunks into one PSUM tile, trailing the cos
    stream by three groups so the PE never waits on the softmax chain.
  - Final Dnew stays [r, d]: Xbar uses it as lhsT with no transpose.
  - ROLLING schedule: batch i+1's setup units fill batch i's D-normalize
    hinges; batch i's Xbar blocks are deferred into batch i+1's early
    step groups so the batch boundary never idles the PE; batch 0's own
    setup interleaves with its first step.  Output copies split ACT/DVE.
  - PSUM: 8 banks = transposes 2 (two-per-bank slot packing) + cos groups
    3 + Xbar 2 + XCt accumulator 1.
  - D-normalize reads a staged SBUF copy of the accumulator: ACT/DVE ops
    that read the XCt PSUM directly diverge on real HW (verified), so the
    chain pays one ACT copy.
"""

import numpy as np

import concourse.bacc as bacc
import concourse.bass as bass
import concourse.mybir as mybir
import concourse.tile as tile
from concourse.bass_utils import run_bass_kernel_spmd

F32 = mybir.dt.float32
BF16 = mybir.dt.bfloat16
AF = mybir.ActivationFunctionType
OP = mybir.AluOpType

N_CORES = 8
B_FULL, D, N, R = 32, 512, 4096, 64
B_LOC = B_FULL // N_CORES          # 4 batches per core
KT = D // 128                      # 4 d-tiles
NC128 = N // 128                   # 32 n-chunks of 128
NB512 = N // 512                   # 8 n-blocks of 512
NG = NB512 // 2                    # 4 block-pairs per step
T_INV = 0.1                        # 1 / temperature
STEPS = 3
EPS2 = 1e-12                       # eps^2 for the norm clamp


def _force_single_act_set():
    """All ACT functions we use (Exp, Ln, Copy) live in the
    natural_log_exp_and_others set; empty the other sets so one table load
    serves the whole kernel."""
    import concourse.hw_specs as hw_specs

    orig = hw_specs.get_activation_tables
    target = "natural_log_exp_and_others"

    def patched(arch):
        t = dict(orig(arch))
        need = {AF.Exp, AF.Ln, AF.Square, AF.Copy}
        if target in t and need <= set(t[target]):
            t = {k: (v if k == target else set()) for k, v in t.items()}
        return t

    bacc.get_activation_tables = patched


class _HalfSlots:
    """Rotating PSUM slots: [p, 2*w] tiles from `pool`, handed out as two
    [p, w] halves so two logical tiles share one 2 KiB bank."""

    def __init__(self, pool, p, w, dtype, tag):
        self.pool, self.p, self.w, self.dtype, self.tag = pool, p, w, dtype, tag
        self.i = 0
        self.cur = None

    def slot(self):
        h = self.i % 2
        if h == 0:
            self.cur = self.pool.tile([self.p, 2 * self.w], self.dtype,
                                      tag=self.tag)
        self.i += 1
        return self.cur[:, h * self.w:(h + 1) * self.w]


class _BatchState:
    def __init__(self):
        self.xbf = None
        self.xt = None
        self.xt2 = None
        self.ssq = None
        self.scl = None
        self.dt_cur = None
        self.dn_bf = None
        self.ct = None
        self.dnew_bf = None
        self.c_r = None


def build_program():
    _force_single_act_set()
    nc = bacc.Bacc()
    x_ext = nc.declare_dram_parameter("X", [B_LOC, D, N], F32, isOutput=False)
    d_ext = nc.declare_dram_parameter("Dinit", [B_LOC, D, R], F32, isOutput=False)
    id_ext = nc.declare_dram_parameter("ident", [128, 128], F32, isOutput=False)
    y_ext = nc.declare_dram_parameter("Y", [B_LOC, D, N], F32, isOutput=True)

    with tile.TileContext(nc) as tc:
        import contextlib

        with contextlib.ExitStack() as ctx:
            singles = ctx.enter_context(tc.tile_pool(name="singles", bufs=1))
            xstage = ctx.enter_context(tc.tile_pool(name="xstage", bufs=4))
            xpool = ctx.enter_context(tc.tile_pool(name="xpool", bufs=2))
            work = ctx.enter_context(tc.tile_pool(name="work", bufs=2))
            sfx = ctx.enter_context(tc.tile_pool(name="sfx", bufs=4))
            dpool = ctx.enter_context(tc.tile_pool(name="dpool", bufs=2))
            opool = ctx.enter_context(tc.tile_pool(name="opool", bufs=4))
            # PSUM banks: tb 3 + pcos 4 + pacc 1 = 8
            ps_tb = ctx.enter_context(tc.tile_pool(name="ps_tb", bufs=2, space="PSUM"))
            ps_cos = ctx.enter_context(tc.tile_pool(name="ps_cos", bufs=2, space="PSUM"))
            ps_acc = ctx.enter_context(tc.tile_pool(name="ps_acc", bufs=1, space="PSUM"))

            tb_slots = _HalfSlots(ps_tb, 128, 512, BF16, "tb")

            id_f = singles.tile([128, 128], F32)
            nc.sync.dma_start(out=id_f, in_=id_ext[:])
            id_b = singles.tile([128, 128], BF16)
            nc.vector.tensor_copy(out=id_b, in_=id_f)
            eps_t = singles.tile([128, 1], F32)
            nc.vector.memset(eps_t, EPS2)

            states = [_BatchState() for _ in range(B_LOC)]

            # ---------------- emission helpers --------------------------
            def emit_alloc_x(b):
                st_ = states[b]
                st_.xbf = [xpool.tile([128, N], BF16, tag=f"xbf{k}",
                                      name=f"xbf{k}_{b}") for k in range(KT)]
                xt2 = [xpool.tile([128, 2 * D], BF16, tag=f"xt{cp}",
                                  name=f"xt{cp}_{b}")
                       for cp in range(NC128 // 2)]
                st_.xt = [xt2[c // 2][:, (c % 2) * D:(c % 2 + 1) * D]
                          for c in range(NC128)]
                st_.xt2 = xt2
                st_.ssq = xpool.tile([128, NC128], F32, tag="ssq")

            def emit_cast_unit(b, k, h):
                """DMA in + cast one [128, 1024] block of X."""
                st_ = states[b]
                stt = xstage.tile([128, N // 4], F32, tag="xst")
                nc.sync.dma_start(
                    out=stt,
                    in_=x_ext[b, k * 128:(k + 1) * 128,
                              h * (N // 4):(h + 1) * (N // 4)],
                )
                dst = st_.xbf[k][:, h * (N // 4):(h + 1) * (N // 4)]
                m = (k + 4 * h) % 4
                if m == 3:
                    nc.scalar.copy(out=dst, in_=stt)
                elif m == 2:
                    nc.gpsimd.tensor_copy(out=dst, in_=stt)
                else:
                    nc.vector.tensor_copy(out=dst, in_=stt)

            def emit_chunk_unit(b, cp):
                """Transpose n-chunk pair cp of X, one [*,1024] copy, and
                per-chunk sum-of-squares (split DVE/ACT)."""
                st_ = states[b]
                sqscr = work.tile([128, D], BF16, tag="scrap")
                pt = ps_tb.tile([128, 1024], BF16, tag="tb")
                for cc in range(2):
                    c = 2 * cp + cc
                    for k in range(KT):
                        nc.tensor.transpose(
                            pt[:, cc * D + k * 128:cc * D + (k + 1) * 128],
                            st_.xbf[k][:, c * 128:(c + 1) * 128],
                            id_b,
                        )
                nc.vector.tensor_copy(out=st_.xt2[cp], in_=pt)
                for cc in range(2):
                    c = 2 * cp + cc
                    if c % 2 == 0:
                        nc.vector.scalar_tensor_tensor(
                            out=sqscr, in0=st_.xt[c], scalar=1.0,
                            in1=st_.xt[c], op0=OP.mult, op1=OP.mult,
                            accum_out=st_.ssq[:, c:c + 1],
                        )
                    else:
                        nc.scalar.activation(
                            out=sqscr, in_=st_.xt[c], func=AF.Square,
                            scale=1.0, bias=0.0,
                            accum_out=st_.ssq[:, c:c + 1],
                        )

            def emit_scl_slice(b, g):
                """scl for chunk-block g (needs only that block's ssq)."""
                st_ = states[b]
                if st_.scl is None:
                    st_.scl = xpool.tile([128, NC128], BF16, tag="scl",
                                         name=f"scl_{b}")
                ln_x = work.tile([128, 8], F32, tag="sclw_ln")
                nc.scalar.activation(out=ln_x,
                                     in_=st_.ssq[:, 8 * g:8 * (g + 1)],
                                     func=AF.Ln, scale=1.0, bias=eps_t[:, 0:1])
                nc.scalar.activation(out=st_.scl[:, 8 * g:8 * (g + 1)],
                                     in_=ln_x, func=AF.Exp, scale=-0.5,
                                     bias=0.0)

            def emit_dinit(b):
                st_ = states[b]
                st_.dt_cur = dpool.tile([64, D], F32, tag="dt", name=f"dt_{b}")
                pdn0 = ps_cos.tile([128, 512], F32, tag="pxb")
                for k in range(KT):
                    dn_nat = work.tile([128, R], F32, tag="dload")
                    nc.sync.dma_start(
                        out=dn_nat, in_=d_ext[b, k * 128:(k + 1) * 128, :]
                    )
                    nc.tensor.transpose(
                        pdn0[0:64, k * 128:(k + 1) * 128], dn_nat, id_f
                    )
                nc.scalar.copy(out=st_.dt_cur, in_=pdn0[0:64, :])

            def setup_units(b):
                """Generator of small setup tasks for batch b."""
                emit_alloc_x(b)
                yield lambda b=b: emit_dinit(b)
                for h in range(4):
                    for k in range(KT):
                        yield lambda b=b, k=k, h=h: emit_cast_unit(b, k, h)
                    for cp in range(4 * h, 4 * h + 4):
                        yield lambda b=b, cp=cp: emit_chunk_unit(b, cp)
                    yield lambda b=b, h=h: emit_scl_slice(b, h)

            def drain(units, k):
                for _ in range(k):
                    u = next(units, None)
                    if u is None:
                        return
                    u()

            def emit_hinge_pre(b, s):
                """DVE/ACT part of the D-column normalize."""
                st_ = states[b]
                ssqd = work.tile([64, 1], F32, tag="ssqd")
                dscr = work.tile([64, D], BF16, tag="scrap")
                nc.vector.scalar_tensor_tensor(
                    out=dscr, in0=st_.dt_cur, scalar=1.0,
                    in1=st_.dt_cur, op0=OP.mult, op1=OP.mult,
                    accum_out=ssqd,
                )
                ln_d = work.tile([64, 1], F32, tag="lnd")
                nc.scalar.activation(out=ln_d, in_=ssqd, func=AF.Ln,
                                     scale=1.0, bias=eps_t[0:64, 0:1])
                rnd = work.tile([64, 1], F32, tag="rnd")
                nc.scalar.activation(out=rnd, in_=ln_d, func=AF.Exp,
                                     scale=-0.5, bias=0.0)
                dnt = work.tile([64, D], BF16, tag="dnt", name=f"dnt_{b}_{s}")
                nc.vector.tensor_scalar_mul(out=dnt, in0=st_.dt_cur,
                                            scalar1=rnd)
                if s == 0:
                    st_.ct = [None] * NG
                return dnt

            def emit_hinge_post(b, s, dnt):
                """PE transposes of Dn^T + ACT copy to dn_bf."""
                st_ = states[b]
                pdn = tb_slots.slot()[:, 0:KT * R]
                for k in range(KT):
                    nc.tensor.transpose(
                        pdn[:, k * R:(k + 1) * R],
                        dnt[:, k * 128:(k + 1) * 128], id_b[0:64, 0:64],
                    )
                st_.dn_bf = work.tile([128, KT, R], BF16, tag="dnbf",
                                      name=f"dnbf_{b}_{s}")
                nc.scalar.copy(
                    out=st_.dn_bf,
                    in_=pdn.rearrange("p (k r) -> p k r", k=KT),
                )

            def emit_cos_softmax(b, s, g):
                """cosT computed directly (X chunk stationary, Dn moving):
                pg[:, ci, :] = cos^T of n-chunk 8g+ci.  Softmax follows in
                [n | r(stride 8), ci(stride 1)] packing so the 1/denominator
                multiply keeps a stride-1 last dim (DVE 2x)."""
                st_ = states[b]
                pg = ps_cos.tile([128, 8, R], F32, tag="pg", bufs=3)
                for ci in range(8):
                    c = 8 * g + ci
                    for k in range(KT):
                        nc.tensor.matmul(
                            pg[:, ci, :],
                            st_.xbf[k][:, c * 128:(c + 1) * 128],
                            st_.dn_bf[:, k, :],
                            start=(k == 0), stop=(k == KT - 1),
                        )
                # logits = cosT * scl, written packed [r(8), ci(1)]
                scl_v = bass.AP(
                    tensor=st_.scl.tensor,
                    offset=st_.scl.offset + 8 * g,
                    ap=[list(st_.scl.ap[0]), [1, 8], [0, R]],
                )
                logits = sfx.tile([128, 512], BF16, tag="logits")
                logits_pack = bass.AP(
                    tensor=logits.tensor, offset=logits.offset,
                    ap=[list(logits.ap[0]), [1, 8], [8, R]],
                )
                nc.vector.tensor_tensor(
                    out=logits_pack, in0=pg, in1=scl_v, op=OP.mult,
                )
                et = sfx.tile([128, 512], BF16, tag="et")
                nc.scalar.activation(
                    out=et, in_=logits, func=AF.Exp, scale=T_INV, bias=0.0,
                )
                et_cr = bass.AP(
                    tensor=et.tensor, offset=et.offset,
                    ap=[list(et.ap[0]), [1, 8], [8, R]],
                )
                s_sum = sfx.tile([128, 8], F32, tag="ssum")
                nc.vector.tensor_reduce(
                    out=s_sum, in_=et_cr, axis=mybir.AxisListType.X,
                    op=OP.add,
                )
                rs_sum = sfx.tile([128, 8], BF16, tag="rssum")
                with nc.allow_low_precision(
                    reason="softmax denominators ~64; bf16 keeps the "
                           "downstream multiply in the DVE 2x mode"
                ):
                    nc.vector.reciprocal(out=rs_sum, in_=s_sum)
                rs_v = bass.AP(
                    tensor=rs_sum.tensor, offset=rs_sum.offset,
                    ap=[list(rs_sum.ap[0]), [0, R], [1, 8]],
                )
                ct = work.tile([128, 512], BF16, tag=f"ct{g}",
                               name=f"ct{g}_{b}_{s}")
                ct_v = bass.AP(
                    tensor=ct.tensor, offset=ct.offset,
                    ap=[list(ct.ap[0]), [8, R], [1, 8]],
                )
                et_v = bass.AP(
                    tensor=et.tensor, offset=et.offset,
                    ap=[list(et.ap[0]), [8, R], [1, 8]],
                )
                nc.vector.tensor_tensor(out=ct_v, in0=et_v, in1=rs_v,
                                        op=OP.mult)
                st_.ct[g] = ct

            def ct_chunk(b, s_unused, c):
                """[p, r] view (stride 8) of ct for global n-chunk c."""
                ct = states[b].ct[c // 8]
                return bass.AP(
                    tensor=ct.tensor, offset=ct.offset + c % 8,
                    ap=[list(ct.ap[0]), [8, R]],
                )

            def emit_xct(b, s, g, pacc):
                st_ = states[b]
                for ci in range(8):
                    c = 8 * g + ci
                    nc.tensor.matmul(
                        pacc, ct_chunk(b, s, c), st_.xt[c],
                        start=(c == 0), stop=(c == NC128 - 1),
                    )

            def emit_cr_pair(b, g, s_dummy=None):
                """Transpose packed CT of pair g back to C[r, 1024-block]."""
                st_ = states[b]
                if st_.c_r is None:
                    st_.c_r = xpool.tile([64, N], BF16, tag="c_r",
                                         name=f"cr_{b}")
                for half in range(2):
                    j = 2 * g + half
                    pcq = tb_slots.slot()[0:64, :]
                    for ci in range(4):
                        nc.tensor.transpose(
                            pcq[:, ci * 128:(ci + 1) * 128],
                            ct_chunk(b, s_dummy, 4 * j + ci), id_b,
                        )
                    nc.vector.tensor_copy(
                        out=st_.c_r[:, j * 512:(j + 1) * 512], in_=pcq
                    )

            def emit_dt_from_pacc(b, s, pacc, final):
                st_ = states[b]
                st_.dt_cur = dpool.tile([64, D], F32, tag="dt",
                                        name=f"dt_{b}_{s}")
                nc.vector.tensor_copy(out=st_.dt_cur, in_=pacc)
                if final:
                    ssqf = work.tile([64, 1], F32, tag="ssqf")
                    fscr = work.tile([64, D], BF16, tag="scrap")
                    nc.scalar.activation(
                        out=fscr, in_=st_.dt_cur, func=AF.Square, scale=1.0,
                        bias=0.0, accum_out=ssqf,
                    )
                    ln_f = work.tile([64, 1], F32, tag="lnf")
                    nc.scalar.activation(out=ln_f, in_=ssqf, func=AF.Ln,
                                         scale=1.0, bias=eps_t[0:64, 0:1])
                    rnf = work.tile([64, 1], F32, tag="rnf")
                    nc.scalar.activation(out=rnf, in_=ln_f, func=AF.Exp,
                                         scale=-0.5, bias=0.0)
                    st_.dnew_bf = work.tile([64, D], BF16, tag="dnewr",
                                            name=f"dnew_{b}")
                    nc.vector.tensor_scalar_mul(out=st_.dnew_bf,
                                                in0=st_.dt_cur, scalar1=rnf)

            def emit_xbar_block(b, jp):
                """Xbar for n-block pair jp: all 4 k-chunks + DMA out."""
                st_ = states[b]
                for k in range(KT):
                    ot = opool.tile([128, 1024], F32, tag="osb")
                    for jj in range(2):
                        j = 2 * jp + jj
                        pxb = ps_cos.tile([128, 512], F32, tag="pxb")
                        nc.tensor.matmul(
                            pxb, st_.dnew_bf[:, k * 128:(k + 1) * 128],
                            st_.c_r[:, j * 512:(j + 1) * 512],
                            start=True, stop=True,
                        )
                        if (k + jj) % 2 == 0:
                            nc.scalar.copy(
                                out=ot[:, jj * 512:(jj + 1) * 512], in_=pxb
                            )
                        else:
                            nc.vector.tensor_copy(
                                out=ot[:, jj * 512:(jj + 1) * 512], in_=pxb
                            )
                    nc.sync.dma_start(
                        out=y_ext[b, k * 128:(k + 1) * 128,
                                  jp * 1024:(jp + 1) * 1024],
                        in_=ot,
                    )

            def emit_batch(b, units_next, tail_prev, units_self=None):
                """Steps for batch b.  Drains setup units of b+1 at hinge
                points and the previous batch's deferred Xbar blocks inside
                the early step-0/1 groups; returns this batch's deferred
                Xbar closures."""
                st_ = states[b]
                for s in range(STEPS):
                    final = s == STEPS - 1
                    pacc = ps_acc.tile([64, D], F32, tag="pacc",
                                       name=f"pacc_{b}_{s}")
                    xq = []
                    for g in range(NG):
                        if s == 0 and g > 0 and units_self is not None:
                            drain(units_self, 10)
                        emit_cos_softmax(b, s, g)
                        if s < 2:
                            drain(tail_prev, 1)
                        else:
                            drain(units_next, 5)
                        xq.append(g)
                        if len(xq) > 3:
                            gg = xq.pop(0)
                            emit_xct(b, s, gg, pacc)
                            if final:
                                emit_cr_pair(b, gg)
                    for gg in xq:
                        emit_xct(b, s, gg, pacc)
                        if final:
                            emit_cr_pair(b, gg)
                    emit_dt_from_pacc(b, s, pacc, final)
                    if not final:
                        drain(units_next, 3)
                        dnt = emit_hinge_pre(b, s + 1)
                        drain(units_next, 9)
                        emit_hinge_post(b, s + 1, dnt)
                drain(units_next, 60)  # exhaust b+1 setup before its steps
                # defer this batch's Xbar into the next batch's early groups;
                # queue the next batch's first hinge now so its ln/exp don't
                # sit behind the Xbar ot-copy flood
                dnt_next = None
                if b + 1 < B_LOC and states[b + 1].dt_cur is not None:
                    dnt_next = emit_hinge_pre(b + 1, 0)

                def xbar_unit(jp):
                    def run():
                        drain(units_next, 4)
                        emit_xbar_block(b, jp)
                    return run

                tail = [xbar_unit(jp) for jp in range(NB512 // 2)]

                def finish():
                    drain(units_next, 50)
                tail.append(finish)
                if dnt_next is not None:
                    emit_hinge_post(b + 1, 0, dnt_next)
                return iter(tail)

            # ---------------- top level ---------------------------------
            u0 = setup_units(0)
            drain(u0, 10)  # dinit + h-block 0 + its scl slice
            dnt0 = emit_hinge_pre(0, 0)
            emit_hinge_post(0, 0, dnt0)
            tail = iter(())
            for b in range(B_LOC):
                units_next = setup_units(b + 1) if b + 1 < B_LOC else iter(())
                tail = emit_batch(b, units_next, tail,
                                  units_self=u0 if b == 0 else None)
            drain(tail, 10)
    nc.finalize()
    return nc


_NC_CACHE = None
_last_in_maps = None


def kernel(X: np.ndarray, D_init: np.ndarray) -> np.ndarray:
    global _NC_CACHE, _last_in_maps
    X = np.asarray(X, dtype=np.float32)
    D_init = np.asarray(D_init, dtype=np.float32)
    if _NC_CACHE is None:
        _NC_CACHE = build_program()
    nc = _NC_CACHE
    ident = np.eye(128, dtype=np.float32)
    in_maps = [
        {
            "X": np.ascontiguousarray(X[i * B_LOC:(i + 1) * B_LOC]),
            "Dinit": np.ascontiguousarray(D_init[i * B_LOC:(i + 1) * B_LOC]),
            "ident": ident,
        }
        for i in range(N_CORES)
    ]
    _last_in_maps = in_maps
    res = run_bass_kernel_spmd(nc, in_maps, list(range(N_CORES)))
    return np.concatenate([res.results[i]["Y"] for i in range(N_CORES)], axis=0)
